# revision 30
# baseline (speedup 1.0000x reference)
"""Detection-loss kernel for Trainium2 (8 NeuronCores, data-parallel over batch).

Reference computes: scatter 64 targets/image into a [B,C,H,W] map + mask,
then masked SmoothL1(preds, map).sum() / num_objects.

The mask is nonzero at <= B*T positions, so the loss only depends on preds
at those positions.  All index math (grid cells, flat gather offsets,
last-writer-wins collision resolution) is a pure function of `targets` and
is done on host; each core *gathers* preds at its 256 target cells with
two 128-descriptor indirect DMAs (channels-last relayout makes each
target's 7 channels one contiguous 28B descriptor), evaluates SmoothL1 in
4 vector ops per group (group 0's chain hides under group 1's DMA
flight), reduces across partitions with a ones-matmul, and returns one
8B row of two f32 partials.

Two gather instructions because the DMA ucode pairs ONE offset per
contiguous destination run: a merged [P, 14]-dest gather with a [P, 2]
offset AP coalesces each partition row into a single 14-element
descriptor and consumes only the first offset per partition.

The output is a single 4B row on purpose: every DMA engine touched by the
final DMA adds a late completion ack (~2-7us for scattered 4B row writes)
that delays the NEFF teardown barrier; a [128, 1] result DMA costs ~5us
of pure epilogue.

Collision losers are pointed at a zero pad element appended to the preds
shard with target value 0, so they contribute exactly 0 loss and no winner
masking is needed on device.  num_objects (the winner count) is a pure
function of `targets`, computed exactly on host.

SmoothL1 via signed clamp (valid for both signs of d, beta=1):
  m = clamp(d, -1, 1);  t = 2d - m;  smoothl1(d) = 0.5*m*t
  (|d|<1: 0.5*d^2 ; d>=1: 0.5*(2d-1)=d-0.5 ; d<=-1: 0.5*(-1)(2d+1)=|d|-0.5)
"""

import numpy as np

B, C, H, W = 32, 7, 400, 400
T = 64
NCORES = 8
BLOC = B // NCORES          # 4 images per core
HW = H * W                  # 160000
CHW = C * HW                # 1120000
NELEM = BLOC * CHW          # 4480000 elements per core
PAD = 8                     # zero pad elements; loser gather slots point here
P = 128
NG = BLOC * T // P          # 2 targets per partition

_cached = {}
TRACE = False  # set True (e.g. from test.py) to capture an NTFF profile


def _build_nc():
    import concourse.bacc as bacc
    import concourse.bass as bass
    import concourse.bass_isa as bass_isa
    import concourse.tile as tile
    import concourse.mybir as mybir

    f32 = mybir.dt.float32
    i32 = mybir.dt.int32
    OP = mybir.AluOpType
    AX = mybir.AxisListType

    nc = bacc.Bacc(
        "TRN2",
        target_bir_lowering=False,
        debug=False,
        enable_asserts=False,
        num_devices=NCORES,
    )

    preds_flat = nc.dram_tensor(
        "preds_flat", [NELEM + PAD, 1], f32, kind="ExternalInput"
    )
    offs_d = nc.dram_tensor("offs", [P, NG], i32, kind="ExternalInput")
    tv_d = nc.dram_tensor("tv", [P, NG * C], f32, kind="ExternalInput")
    out_d = nc.dram_tensor("out", [4, 32], f32, kind="ExternalOutput")

    with tile.TileContext(nc) as tc:
        with tc.tile_pool(name="sbuf", bufs=1) as sb:
            offs = sb.tile([P, NG], i32)
            nc.sync.dma_start(offs[:], offs_d[:, :], single_packet=True)
            tv = sb.tile([P, NG * C], f32)
            nc.sync.dma_start(tv[:], tv_d[:, :], single_packet=True)

            # two 128-descriptor gathers (the HW pairs one offset per
            # contiguous dest run, so each group needs its own instruction)
            gat = sb.tile([P, NG * C], f32)
            for g in range(NG):
                nc.gpsimd.indirect_dma_start(
                    out=gat[:, g * C : (g + 1) * C],
                    out_offset=None,
                    in_=preds_flat[:, :],
                    in_offset=bass.IndirectOffsetOnAxis(
                        ap=offs[:, g : g + 1], axis=0
                    ),
                )

            # one [P, 14] chain over both groups (op cost is width-
            # independent at this size, and the chain has to wait for the
            # second gather anyway); row-sum accumulates into lossq col 0
            lossq = sb.tile([P, 32], f32)
            d = sb.tile([P, NG * C], f32)
            nc.vector.tensor_sub(d[:], gat[:, :], tv[:, :])
            m = sb.tile([P, NG * C], f32)
            nc.vector.tensor_scalar(m[:], d[:], 1.0, -1.0, OP.min, OP.max)
            t = sb.tile([P, NG * C], f32)
            nc.vector.scalar_tensor_tensor(
                t[:], d[:], 2.0, m[:], OP.mult, OP.subtract
            )
            scr = sb.tile([P, NG * C], f32)
            nc.vector.scalar_tensor_tensor(
                scr[:], t[:], 0.5, m[:], OP.mult, OP.mult,
                accum_out=lossq[:, 0:1],
            )

            # cross-partition reduction without leaving the DVE: 32x32 block
            # transpose puts the 128 partials into rows 0/32/64/96 (cols
            # 1-31 of lossq are never initialized; their transposed garbage
            # lands only in rows we never read).  DMA those 4 rows (4x32
            # partials); the host adds the 128 values per core.
            tp = sb.tile([P, 32], f32)
            nc.vector.transpose(tp[:], lossq[:, :])
            nc.sync.dma_start(out_d[:, :], tp[0:P:32, :])

    nc.compile()
    return nc


def _get_nc():
    if "nc" not in _cached:
        _cached["nc"] = _build_nc()
    return _cached["nc"]


def _make_in_maps(preds, targets):
    # grid coords in float32 semantics matching the reference exactly
    txy = targets[:, :, 0:2].astype(np.float32)
    g = np.floor(txy * np.float32(W / 80.0))
    gx = np.clip(g[:, :, 0], 0, W - 1).astype(np.int64)
    gy = np.clip(g[:, :, 1], 0, H - 1).astype(np.int64)

    # channels-last relayout so each target's 7 channels are one contiguous
    # 28B indirect-DMA descriptor
    preds_t = preds.transpose(0, 2, 3, 1)  # [B, H, W, C]

    jj = np.repeat(np.arange(BLOC), T)  # local image id per slot n
    in_maps = []
    num_objects = 0
    for k in range(NCORES):
        pshard = np.empty((NELEM + PAD, 1), np.float32)
        pshard[:NELEM, 0] = preds_t[k * BLOC : (k + 1) * BLOC].reshape(-1)
        pshard[NELEM:, 0] = 0.0

        sl = slice(k * BLOC, (k + 1) * BLOC)
        toff = (
            jj * CHW + (gy[sl].reshape(-1) * W + gx[sl].reshape(-1)) * C
        )  # [256] int64 flat offsets
        tval = targets[sl].reshape(BLOC * T, C).astype(np.float32).copy()

        # last-writer-wins: for duplicate cells (only possible within an
        # image; j is baked into the offset) the largest slot n wins,
        # matching jax scatter .set semantics over the [B, T] update order
        last = {}
        for n in range(BLOC * T):
            last[int(toff[n])] = n
        win = np.zeros(BLOC * T, bool)
        win[list(last.values())] = True
        num_objects += len(last)
        toff[~win] = NELEM  # gather the zero pad element
        tval[~win] = 0.0    # d = 0 - 0 -> zero loss contribution

        in_maps.append(
            {
                "preds_flat": pshard,
                "offs": np.ascontiguousarray(toff.reshape(P, NG).astype(np.int32)),
                "tv": np.ascontiguousarray(tval.reshape(P, NG * C)),
            }
        )
    return in_maps, num_objects


def kernel(preds, targets):
    from concourse.bass_utils import run_bass_kernel_spmd

    preds = np.ascontiguousarray(np.asarray(preds), dtype=np.float32)
    targets = np.ascontiguousarray(np.asarray(targets), dtype=np.float32)
    assert preds.shape == (B, C, H, W) and targets.shape == (B, T, C)

    nc = _get_nc()
    in_maps, num_objects = _make_in_maps(preds, targets)
    res = run_bass_kernel_spmd(nc, in_maps, list(range(NCORES)), trace=TRACE)
    _cached["last_results"] = res

    lsum = np.float64(0.0)
    for k in range(NCORES):
        lsum += res.results[k]["out"].astype(np.float64).sum()
    loss = np.float32(lsum / (np.float64(num_objects) + 1e-6))
    return loss, np.float32(num_objects)


# revision 33
# speedup vs baseline: 1.0212x; 1.0212x over previous
"""Detection-loss kernel for Trainium2 (8 NeuronCores, data-parallel over batch).

Reference computes: scatter 64 targets/image into a [B,C,H,W] map + mask,
then masked SmoothL1(preds, map).sum() / num_objects.

The mask is nonzero at <= B*T positions, so the loss only depends on preds
at those positions.  All index math (grid cells, flat gather offsets,
last-writer-wins collision resolution) is a pure function of `targets` and
is done on host; each core *gathers* preds at its 256 target cells with
two 128-descriptor indirect DMAs (channels-last relayout makes each
target's 7 channels one contiguous 28B descriptor), evaluates SmoothL1 in
one 4-op [P, 14] vector chain with a fused row-sum, folds the 128
per-partition partials into 4 rows with a DVE 32x32 stream transpose, and
DMAs those 4 rows out.  The host adds 128 values per core.

Two gather instructions because the DMA ucode pairs ONE offset per
contiguous destination run: a merged [P, 14]-dest gather with a [P, 2]
offset AP coalesces each partition row into a single 14-element
descriptor and consumes only the first offset per partition.

The output is 4 rows on purpose: every DMA engine touched by the final
DMA adds a late completion ack (~2-7us for scattered 4B row writes) that
delays the NEFF teardown; a [128, 1] result DMA (16 engines) costs ~5us
of pure epilogue.  The transpose also keeps the whole reduction on the
DVE - a PE ones-matmul works but pays two cross-engine handoffs and a
PSUM->SBUF copy.

Collision losers are pointed at a zero pad element appended to the preds
shard with target value 0, so they contribute exactly 0 loss and no winner
masking is needed on device.  num_objects (the winner count) is a pure
function of `targets`, computed exactly on host.

SmoothL1 via signed clamp (valid for both signs of d, beta=1):
  m = clamp(d, -1, 1);  t = 2d - m;  smoothl1(d) = 0.5*m*t
  (|d|<1: 0.5*d^2 ; d>=1: 0.5*(2d-1)=d-0.5 ; d<=-1: 0.5*(-1)(2d+1)=|d|-0.5)
"""

import numpy as np

B, C, H, W = 32, 7, 400, 400
T = 64
NCORES = 8
BLOC = B // NCORES          # 4 images per core
HW = H * W                  # 160000
CHW = C * HW                # 1120000
NELEM = BLOC * CHW          # 4480000 elements per core
PAD = 8                     # zero pad elements; loser gather slots point here
P = 128
NG = BLOC * T // P          # 2 targets per partition

_cached = {}
TRACE = False  # set True (e.g. from test.py) to capture an NTFF profile


def _build_nc():
    import concourse.bacc as bacc
    import concourse.bass as bass
    import concourse.tile as tile
    import concourse.mybir as mybir

    f32 = mybir.dt.float32
    i32 = mybir.dt.int32
    OP = mybir.AluOpType

    nc = bacc.Bacc(
        "TRN2",
        target_bir_lowering=False,
        debug=False,
        enable_asserts=False,
        num_devices=NCORES,
    )

    preds_flat = nc.dram_tensor(
        "preds_flat", [NELEM + PAD, 1], f32, kind="ExternalInput"
    )
    offs_d = nc.dram_tensor("offs", [P, NG], i32, kind="ExternalInput")
    tv_d = nc.dram_tensor("tv", [P, NG * C], f32, kind="ExternalInput")
    out_d = nc.dram_tensor("out", [4, 32], f32, kind="ExternalOutput")

    with tile.TileContext(nc) as tc:
        with tc.tile_pool(name="sbuf", bufs=1) as sb:
            offs = sb.tile([P, NG], i32)
            nc.sync.dma_start(offs[:], offs_d[:, :])
            tv = sb.tile([P, NG * C], f32)
            nc.sync.dma_start(tv[:], tv_d[:, :])

            # two 128-descriptor gathers (the HW pairs one offset per
            # contiguous dest run, so each group needs its own instruction)
            gat = sb.tile([P, NG * C], f32)
            for g in range(NG):
                nc.gpsimd.indirect_dma_start(
                    out=gat[:, g * C : (g + 1) * C],
                    out_offset=None,
                    in_=preds_flat[:, :],
                    in_offset=bass.IndirectOffsetOnAxis(
                        ap=offs[:, g : g + 1], axis=0
                    ),
                )

            # one [P, 14] chain over both groups (op cost is width-
            # independent at this size, and the chain has to wait for the
            # second gather anyway); row-sum accumulates into lossq col 0
            lossq = sb.tile([P, 32], f32)
            d = sb.tile([P, NG * C], f32)
            nc.vector.tensor_sub(d[:], gat[:, :], tv[:, :])
            m = sb.tile([P, NG * C], f32)
            nc.vector.tensor_scalar(m[:], d[:], 1.0, -1.0, OP.min, OP.max)
            t = sb.tile([P, NG * C], f32)
            nc.vector.scalar_tensor_tensor(
                t[:], d[:], 2.0, m[:], OP.mult, OP.subtract
            )
            scr = sb.tile([P, NG * C], f32)
            nc.vector.scalar_tensor_tensor(
                scr[:], t[:], 0.5, m[:], OP.mult, OP.mult,
                accum_out=lossq[:, 0:1],
            )

            # cross-partition reduction without leaving the DVE: 32x32 block
            # transpose puts the 128 partials into rows 0/32/64/96 (cols
            # 1-31 of lossq are never initialized; their transposed garbage
            # lands only in rows we never read).  DMA those 4 rows (4x32
            # partials); the host adds the 128 values per core.
            tp = sb.tile([P, 32], f32)
            nc.vector.transpose(tp[:], lossq[:, :])
            nc.sync.dma_start(out_d[:, :], tp[0:P:32, :])

    nc.compile()
    return nc


def _get_nc():
    if "nc" not in _cached:
        _cached["nc"] = _build_nc()
    return _cached["nc"]


def _make_in_maps(preds, targets):
    # grid coords in float32 semantics matching the reference exactly
    txy = targets[:, :, 0:2].astype(np.float32)
    g = np.floor(txy * np.float32(W / 80.0))
    gx = np.clip(g[:, :, 0], 0, W - 1).astype(np.int64)
    gy = np.clip(g[:, :, 1], 0, H - 1).astype(np.int64)

    # channels-last relayout so each target's 7 channels are one contiguous
    # 28B indirect-DMA descriptor
    preds_t = preds.transpose(0, 2, 3, 1)  # [B, H, W, C]

    jj = np.repeat(np.arange(BLOC), T)  # local image id per slot n
    in_maps = []
    num_objects = 0
    for k in range(NCORES):
        pshard = np.empty((NELEM + PAD, 1), np.float32)
        pshard[:NELEM, 0] = preds_t[k * BLOC : (k + 1) * BLOC].reshape(-1)
        pshard[NELEM:, 0] = 0.0

        sl = slice(k * BLOC, (k + 1) * BLOC)
        toff = (
            jj * CHW + (gy[sl].reshape(-1) * W + gx[sl].reshape(-1)) * C
        )  # [256] int64 flat offsets
        tval = targets[sl].reshape(BLOC * T, C).astype(np.float32).copy()

        # last-writer-wins: for duplicate cells (only possible within an
        # image; j is baked into the offset) the largest slot n wins,
        # matching jax scatter .set semantics over the [B, T] update order
        last = {}
        for n in range(BLOC * T):
            last[int(toff[n])] = n
        win = np.zeros(BLOC * T, bool)
        win[list(last.values())] = True
        num_objects += len(last)
        toff[~win] = NELEM  # gather the zero pad element
        tval[~win] = 0.0    # d = 0 - 0 -> zero loss contribution

        in_maps.append(
            {
                "preds_flat": pshard,
                "offs": np.ascontiguousarray(toff.reshape(P, NG).astype(np.int32)),
                "tv": np.ascontiguousarray(tval.reshape(P, NG * C)),
            }
        )
    return in_maps, num_objects


def kernel(preds, targets):
    from concourse.bass_utils import run_bass_kernel_spmd

    preds = np.ascontiguousarray(np.asarray(preds), dtype=np.float32)
    targets = np.ascontiguousarray(np.asarray(targets), dtype=np.float32)
    assert preds.shape == (B, C, H, W) and targets.shape == (B, T, C)

    nc = _get_nc()
    in_maps, num_objects = _make_in_maps(preds, targets)
    res = run_bass_kernel_spmd(nc, in_maps, list(range(NCORES)), trace=TRACE)
    _cached["last_results"] = res

    lsum = np.float64(0.0)
    for k in range(NCORES):
        lsum += res.results[k]["out"].astype(np.float64).sum()
    loss = np.float32(lsum / (np.float64(num_objects) + 1e-6))
    return loss, np.float32(num_objects)
